# revision 26
# baseline (speedup 1.0000x reference)
"""Trainium2 Bass kernel for nn_MultiMPNN (gnn_message_passing).

Reference computation (B=4, N=512, Z=64, E=16, H=128):
    msgs[b,i,j,:] = z[b,i]@W_i + z[b,j]@W_j + e_feat[b,i,j]@W_e + b_msg
    agg[b,i,:]    = max_j (msgs + (adj>0 ? 0 : -inf))
    out           = z@Wu_z + agg@Wu_h + b_upd

Design (v2):
 1. Everything under the max folds into ONE matmul per destination row with
    contraction K = E + Z = 80:
      lhsT_aug[80,128] = [W_e ; W_j]                      (constant, fp8)
      rhs_aug [80,w]   = [e_feat[b,i,sel].T ; z[b,sel].T] (streamed, fp8)
      PSUM[h,j] = ze + zj  ->  max over j -> agg column
    zi + b_msg commute out of the max; z@Wu_z + b_upd is computed on the
    host (tiny, exact f32).  No mask row: the host streams only active j's
    and pads each row to its width with DUPLICATES of an active column
    (max unchanged).
 2. fp8 e4m3 stream + weights with MatmulPerfMode.DoubleRow ([40,2,*]
    k-tiles): halves both HBM bytes and PE cycles vs bf16.
 3. Global load balance: all B*N rows are sorted by active-edge count and
    dealt round-robin to the 8 cores, so the shared per-group widths are
    (nearly) exact sorted counts instead of per-core maxima.
 4. The PSUM drain (the bottleneck: every message must be touched once in
    f32 PSUM) is spread across THREE engines per 4-row group:
      lane A: ACT copies PSUM -> bf16 SBUF; residue max-tree on DVE (2x
              bf16 mode) or GPSIMD (tensor_tensor max), final small
              reduce_max on DVE.
      lane D: DVE tensor_tensor(max) folds the two halves of each PSUM row
              directly (f32, one pass) into a bf16 residue of w/2; GPSIMD
              tree + DVE reduce finish it.
    The LANE_PATTERN mix balances DVE/ACT/GPSIMD busy time.
"""

import numpy as np
import ml_dtypes

import concourse.bacc as bacc
import concourse.mybir as mybir
import concourse.tile as tile
from concourse import bass_utils
from concourse.bass_interp import get_hw_module
from contextlib import ExitStack

# NB: walrus's --enable-ldw-opt=true crashes codegen on DoubleRow
# LDWEIGHTS, so the per-matmul weight reload (~200ns) stays.

B, N, Z, E, H = 4, 512, 64, 16, 128
NCORES = 8
IH = N * B // NCORES          # 256 destination rows per core
K = E + Z                     # 80 contraction rows (no mask row)
KP = K // 2                   # 40 partitions x 2 k-tiles (DoubleRow)
RG = 4                        # rows per PSUM tile / drain group
NG = IH // RG                 # 64 groups
BANK = 512                    # f32 elems per PSUM bank

F32 = mybir.dt.float32
BF16 = mybir.dt.bfloat16
FP8 = mybir.dt.float8e4
NP_FP8 = ml_dtypes.float8_e4m3

# Drain lane per group, cycled:  (NB: a DVE/GPSIMD op may read at most ONE
# non-scalar input from PSUM, so pairwise max directly on PSUM is illegal.)
#   "R":  single DVE reduce_max straight from PSUM (one op, f32 rate)
#   "AD": ACT copy + DVE tree + DVE reduce
#   "AG": ACT copy + GPSIMD tree (2 lvls) + DVE reduce
# Walrus rejects TENSOR_TENSOR on the Pool/GPSIMD engine, so only DVE and
# ACT can touch the reduction; ~4/5 of groups go through the ACT copy.
LANE_PATTERN = ["AD", "AD", "R", "AD", "AD"]

# Row-blocks (in groups of RG rows) per stream DMA; small first blocks so
# compute starts early.
BLOCK_GROUPS = [1, 1, 2, 4, 8, 8, 8, 8, 8, 8, 8]

TRACE = False                 # test.py sets True to capture an NTFF profile
TRACE_DIR = None              # optional fixed dir for trace artifacts
LAST_RESULTS = None           # BassKernelResults of the last run (for test.py)

_MODULE_CACHE = {}


def _ensure_ntff_hook():
    """The agent image's antenv lacks axon_hooks; recreate it so
    run_bass_kernel_spmd(trace=True) can reach the axon NTFF profiler."""
    import sys
    import types

    try:
        import antenv.axon_hooks  # noqa: F401

        return
    except ImportError:
        pass
    import antenv
    from trn_agent_boot.trn_boot import _ntff_profile_via_ctypes

    state = {"h": _ntff_profile_via_ctypes("/opt/axon/libaxon_pjrt.so")}
    mod = types.ModuleType("antenv.axon_hooks")
    mod.get_axon_ntff_profile_hook = lambda: state["h"]
    mod.set_axon_ntff_profile_hook = lambda h: state.__setitem__("h", h)
    sys.modules["antenv.axon_hooks"] = mod
    antenv.axon_hooks = mod


def _build_module(widths):
    widths = [int(w) for w in widths]          # per-group widths, len NG
    row_w = [w for w in widths for _ in range(RG)]
    offs = [0]
    for w in row_w:
        offs.append(offs[-1] + w)
    tot = offs[-1]

    nc = bacc.Bacc(
        "TRN2",
        target_bir_lowering=False,
        debug=False,
        enable_asserts=False,
        num_devices=NCORES,
    )

    # Column-interleaved k-tiles: per partition, column c's two k-tile
    # bytes are adjacent ([KP, tot, 2] flattened).  Keeps the matmul rhs
    # AP strides tiny (the whole-stream k-tile stride would overflow the
    # 16-bit ISA step field).
    stream = nc.dram_tensor("stream", [KP, 2 * tot], FP8, kind="ExternalInput")
    lhst = nc.dram_tensor("lhst", [KP, 2, H], FP8, kind="ExternalInput")
    zit = nc.dram_tensor("zit", [H, IH], F32, kind="ExternalInput")
    hostc = nc.dram_tensor("hostc", [H, IH], F32, kind="ExternalInput")
    wuh = nc.dram_tensor("wuh", [H, H], F32, kind="ExternalInput")
    ident = nc.dram_tensor("ident", [H, H], F32, kind="ExternalInput")
    out = nc.dram_tensor("out", [IH, H], F32, kind="ExternalOutput")

    with ExitStack() as ctx:
        tc = ctx.enter_context(tile.TileContext(nc))
        const = ctx.enter_context(tc.tile_pool(name="const", bufs=1))
        psum = ctx.enter_context(tc.tile_pool(name="psum", bufs=2, space="PSUM"))
        stage = ctx.enter_context(tc.tile_pool(name="stage", bufs=8))

        lhst_sb = const.tile([KP, 2 * H], FP8, tag="lhst")
        nc.scalar.dma_start(
            lhst_sb[:, :].rearrange("p (t c) -> p t c", t=2), lhst.ap()
        )
        zit_sb = const.tile([H, IH], F32, tag="zit")
        nc.scalar.dma_start(zit_sb[:, :], zit.ap())
        hostc_sb = const.tile([H, IH], F32, tag="hostc")
        nc.scalar.dma_start(hostc_sb[:, :], hostc.ap())
        wuh_sb = const.tile([H, H], F32, tag="wuh")
        nc.scalar.dma_start(wuh_sb[:, :], wuh.ap())
        ident_sb = const.tile([H, H], F32, tag="ident")
        nc.scalar.dma_start(ident_sb[:, :], ident.ap())

        mega = const.tile([KP, 2 * tot], FP8, tag="mega")
        lhst3 = lhst_sb[:, :].rearrange("p (t c) -> p t c", t=2)

        # bf16 so the DVE reduce/tree ops qualify for 2x mode (a single f32
        # operand drops the whole op to 1x).
        magg = const.tile([H, IH], BF16, tag="magg")

        # PE warm-up: the clock gate keeps the PE slow until it has been
        # busy for a while; burn the DMA-dominated startup window.
        warm_a = const.tile([H, BANK], BF16, tag="warm_a")
        nc.vector.memset(warm_a[:, :], 0.0)
        pw = psum.tile([H, RG * BANK], F32, tag="ps")
        for _ in range(6):
            nc.tensor.matmul(
                pw[:, :BANK], warm_a[:, :H], warm_a[:, :], start=True, stop=True
            )

        stream_ap = stream.ap()
        g0 = 0
        for blk, ngrp in enumerate(BLOCK_GROUPS):
            c0, c1 = offs[g0 * RG], offs[(g0 + ngrp) * RG]
            nc.sync.dma_start(mega[:, 2 * c0 : 2 * c1], stream_ap[:, 2 * c0 : 2 * c1])
            # Within a block the two k-tiles are stored as [2, blkcols]
            # (contiguous columns) so the matmul rhs innermost stride is 1.
            mb3 = mega[:, 2 * c0 : 2 * c1].rearrange("p (t c) -> p t c", t=2)

            for gi in range(g0, g0 + ngrp):
                w = widths[gi]
                w2, w4 = w // 2, w // 4
                i0 = gi * RG
                lane = LANE_PATTERN[gi % len(LANE_PATTERN)]

                ps = psum.tile([H, RG * BANK], F32, tag="ps")
                psv = ps[:, :].rearrange("p (b j) -> p b j", b=RG)
                paired = w <= BANK // 2
                if paired:
                    # Two rows back-to-back per bank/matmul: halves the
                    # LDWEIGHTS count (the PE reloads lhsT per matmul).
                    for h in range(RG // 2):
                        o = offs[i0 + 2 * h] - c0
                        nc.tensor.matmul(
                            psv[:, h, : 2 * w],
                            lhst3[:, :, :],
                            mb3[:, :, o : o + 2 * w],
                            start=True,
                            stop=True,
                            perf_mode=mybir.MatmulPerfMode.DoubleRow,
                        )
                    ps_src = psv[:, : RG // 2, : 2 * w]
                    ps_rows = ps_src.rearrange("p b (r j) -> p b r j", r=2)
                else:
                    for r in range(RG):
                        o = offs[i0 + r] - c0
                        nc.tensor.matmul(
                            psv[:, r, :w],
                            lhst3[:, :, :],
                            mb3[:, :, o : o + w],
                            start=True,
                            stop=True,
                            perf_mode=mybir.MatmulPerfMode.DoubleRow,
                        )
                    ps_rows = psv[:, :, :w]
                    ps_src = ps_rows

                if lane == "R":
                    nc.vector.reduce_max(
                        magg[:, i0 : i0 + RG], ps_rows, axis=mybir.AxisListType.X
                    )
                    continue

                # ACT drains the group to bf16 SBUF; tree-max the residue.
                av = stage.tile([H, RG * w], BF16, tag="astage")
                nc.scalar.copy(
                    av[:, :].rearrange("p (b j) -> p b j", b=ps_src.shape[1]),
                    ps_src,
                )
                av3 = av[:, :].rearrange("p (b j) -> p b j", b=RG)
                lvl, lw = av3, w

                eng = nc.vector if lane[1] == "D" else nc.gpsimd
                while lw > w4:
                    nw = lw // 2
                    ht = stage.tile([H, RG * nw], BF16, tag="tree")
                    ht3 = ht[:, :].rearrange("p (b j) -> p b j", b=RG)
                    eng.tensor_tensor(
                        ht3[:, :, :],
                        lvl[:, :, :nw],
                        lvl[:, :, nw : 2 * nw],
                        mybir.AluOpType.max,
                    )
                    lvl, lw = ht3, nw
                nc.vector.reduce_max(
                    magg[:, i0 : i0 + RG], lvl[:, :, :], axis=mybir.AxisListType.X
                )
            g0 += ngrp

        # out = Wu_h.T @ (magg + zi + b_msg) + (z@Wu_z + b_upd), in two
        # column halves so the tail overlaps the last drains.
        aggt = const.tile([H, IH], F32, tag="aggt")
        outt = const.tile([H, IH], F32, tag="outt")
        out_ap = out.ap()
        for t in range(IH // H):
            sl = slice(t * H, (t + 1) * H)
            nc.vector.tensor_add(aggt[:, sl], magg[:, sl], zit_sb[:, sl])
            psf = psum.tile([H, RG * BANK], F32, tag="ps")
            nc.tensor.matmul(
                psf[:, :H], wuh_sb[:, :], aggt[:, sl], start=True, stop=True
            )
            nc.vector.tensor_add(outt[:, sl], psf[:, :H], hostc_sb[:, sl])
            pst = psum.tile([H, RG * BANK], F32, tag="ps")
            nc.tensor.transpose(pst[:, :H], outt[:, sl], ident_sb[:, :])
            osb = const.tile([H, H], F32, tag=f"osb{t}")
            nc.scalar.copy(osb[:, :], pst[:, :H])
            nc.sync.dma_start(out_ap[sl, :], osb[:, :])

    nc.compile()
    nc.m = get_hw_module(nc.m)
    return nc


def _prepare(z, e_feat, adj, W_msg, b_msg, W_upd, b_upd):
    """Host-side global row balancing + per-row compaction.

    All B*N destination rows are sorted by active-edge count (desc) and
    dealt round-robin to cores; group widths are then shared across cores
    by construction.  Returns (in_maps, widths, assign) where assign[c][r]
    is the flat (b*N+i) row id owned by core c at local slot r.
    """
    W_i, W_j, W_e = W_msg[:Z], W_msg[Z : 2 * Z], W_msg[2 * Z :]
    Wu_z, Wu_h = W_upd[:Z], W_upd[Z:]

    adj_b = adj.reshape(B * N, N) > 0
    counts = adj_b.sum(axis=-1)
    assert counts.min() > 0, "isolated destination row: reference output is -inf"
    glob = np.argsort(-counts, kind="stable")          # [B*N] sorted row ids
    # widths per group g (shared): max count among global ranks
    # [32g, 32g+32) = count at rank 32g (desc order), rounded to mult of 4.
    gmax = counts[glob[:: RG * NCORES]]
    widths = np.maximum((gmax + 7) // 8 * 8, 16).astype(int)  # [NG], mult of 8
    row_w = np.repeat(widths, RG)
    offs = np.concatenate([[0], np.cumsum(row_w)])
    tot = int(offs[-1])

    w_aug = np.concatenate([W_e, W_j], axis=0).astype(NP_FP8)  # [K, H]
    lhst_np = np.ascontiguousarray(
        w_aug.reshape(2, KP, H).transpose(1, 0, 2)
    )                                                          # [KP, 2, H]
    wuh_np = np.ascontiguousarray(Wu_h, np.float32)
    ident_np = np.eye(H, dtype=np.float32)

    zf8 = z.reshape(B * N, Z).astype(NP_FP8)
    ef8 = e_feat.reshape(B * N, N, E)

    assign = []
    in_maps = []
    for c in range(NCORES):
        rows = glob[c::NCORES]                       # [IH] flat row ids
        assign.append(rows)
        sflat = np.empty((K, tot), dtype=NP_FP8)
        for r in range(IH):
            w = row_w[r]
            o = offs[r]
            fid = rows[r]
            b, i = divmod(fid, N)
            jsel = np.flatnonzero(adj_b[fid])
            jj = np.empty(w, np.int64)
            jj[: len(jsel)] = jsel[:w]
            jj[len(jsel) :] = jsel[0]                # pad = duplicate column
            sflat[:E, o : o + w] = ef8[fid][jj].T.astype(NP_FP8)
            sflat[E:, o : o + w] = zf8[b * N + jj].T
        # [K, tot] -> per-DMA-block [KP, 2, blkcols]: row k = k-tile
        # (k // KP) + partition (k % KP); within a block the k-tiles are
        # stored contiguously so matmul rhs APs have innermost stride 1
        # and a k-tile stride that fits the 16-bit ISA step field.
        stream = np.empty((KP, 2 * tot), dtype=NP_FP8)
        g0 = 0
        for ngrp in BLOCK_GROUPS:
            c0, c1 = int(offs[g0 * RG]), int(offs[(g0 + ngrp) * RG])
            blk = c1 - c0
            stream[:, 2 * c0 : 2 * c1] = (
                sflat[:, c0:c1].reshape(2, KP, blk).transpose(1, 0, 2)
            ).reshape(KP, 2 * blk)
            g0 += ngrp

        zperm = z.reshape(B * N, Z)[rows]
        in_maps.append(
            {
                "stream": stream,
                "lhst": lhst_np,
                "zit": np.ascontiguousarray(
                    (zperm @ W_i).T + b_msg[:, None], dtype=np.float32
                ),
                "hostc": np.ascontiguousarray(
                    (zperm @ Wu_z + b_upd).T, dtype=np.float32
                ),
                "wuh": wuh_np,
                "ident": ident_np,
            }
        )
    return in_maps, widths, assign


def kernel(z, e_feat, adj, W_msg, b_msg, W_upd, b_upd):
    global LAST_RESULTS

    z = np.asarray(z, np.float32)
    e_feat = np.asarray(e_feat, np.float32)
    adj = np.asarray(adj)
    W_msg = np.asarray(W_msg, np.float32)
    b_msg = np.asarray(b_msg, np.float32)
    W_upd = np.asarray(W_upd, np.float32)
    b_upd = np.asarray(b_upd, np.float32)

    in_maps, widths, assign = _prepare(z, e_feat, adj, W_msg, b_msg, W_upd, b_upd)

    key = tuple(widths)
    if key not in _MODULE_CACHE:
        _MODULE_CACHE[key] = _build_module(widths)
    nc = _MODULE_CACHE[key]

    if TRACE:
        _ensure_ntff_hook()
    res = bass_utils.run_bass_kernel_spmd(
        nc, in_maps, core_ids=list(range(NCORES)), trace=TRACE, tmpdir=TRACE_DIR
    )
    LAST_RESULTS = res

    full = np.empty((B * N, H), np.float32)
    for c in range(NCORES):
        full[assign[c]] = res.results[c]["out"]
    return full.reshape(B, N, H)


if __name__ == "__main__":
    rng = np.random.default_rng(0)
    ins = {
        "z": rng.standard_normal((B, N, Z)).astype(np.float32),
        "e_feat": rng.standard_normal((B, N, N, E)).astype(np.float32),
        "adj": (rng.random((B, N, N)) < 0.5).astype(np.int32),
        "W_msg": (rng.standard_normal((2 * Z + E, H)) * 0.1).astype(np.float32),
        "b_msg": np.zeros(H, np.float32),
        "W_upd": (rng.standard_normal((Z + H, H)) * 0.1).astype(np.float32),
        "b_upd": np.zeros(H, np.float32),
    }
    out = kernel(**ins)
    print("out", out.shape, out.dtype, float(np.abs(out).max()))
